# revision 2
# baseline (speedup 1.0000x reference)
"""Trainium2 Bass kernel v2 for the dual-stream conv + cross-width attention.

Key idea vs v1: the PE pays ~270ns whenever consecutive matmuls change (K, M)
geometry; uniform-geometry streams run ~2.3x faster (187-202ns @ N=388 vs
460ns).  So every phase uses one matmul shape:

  conv1/conv2/conv3: all (K=128, M=128, N=388).  conv1 packs taps as
    3x dy01-pairs + 3x dy2 (upper-half-zero weights); conv2/conv3 are
    block-diagonal over the two feature-extractor streams (9 taps each),
    with inputs stream-packed [fe1 ci | fe2 ci] on partitions.
  scores: f16 Q/K, (K=64, M=128, N=192) x4 then (K=64, M=64, N=192) x4.
  applies: 4 classes of 4 mms: (128,128,64), (128,64,64), (64,128,64),
    (64,64,64), software-pipelined over rows (stage1 = groups se0/se1 of
    row r, stage2 = se2/se3 of row r-1) so PE never waits on exp/scale.

Sharding: 8 cores = (batch b, H-half), each computes 96 rows of all outputs.
"""

import sys

sys.path.insert(0, "/opt/trn_rl_repo")

from contextlib import ExitStack

import numpy as np
import ml_dtypes

import bass_rust
import concourse.bass as bass
import concourse.bacc as bacc
import concourse.mybir as mybir
from concourse import tile
from concourse.vector_clock import ScopedClock
from concourse import tile_utils

# ----------------------------------------------------------------------------
# Workaround: walrus in this container rejects Drain instructions with >1 sem
# wait. Split the TileContext tail drain into one-wait-per-Drain.
def _patched_drain_and_barrier(self, tick_clock, wait_clock):
    nc = self.nc
    drain_inst = nc.sync.drain()
    wait_clock.add_sem_waits(
        drain_inst.ins, ScopedClock({None: tick_clock.global_clock})
    )
    si = drain_inst.ins.sync_info
    if si is not None and len(si.on_wait) > 1:
        waits = list(si.on_wait)
        drain_inst.ins.sync_info = bass_rust.SyncInfo(
            on_wait=[waits[0]], on_update=list(si.on_update)
        )
        for w in waits[1:]:
            d = nc.sync.drain()
            d.ins.sync_info = bass_rust.SyncInfo(on_wait=[w], on_update=[])
    nc.all_engine_barrier()
    assert self.sems is not None
    popped = nc._tile_sem_poison_stack.pop()
    assert popped is self._sem_poison
    nc.clear_and_free_semaphores(list(self.sems.allocated().values()))
    nc.all_engine_barrier()


tile.TileContext._drain_and_barrier = _patched_drain_and_barrier
tile_utils.max_sbuf_usage = 206 * 1024

# ----------------------------------------------------------------------------
B, C, H, W = 4, 64, 192, 192
NCORES = 8
HLOC = 96
R = 12
NBLK = HLOC // R
WP = 194
F32 = mybir.dt.float32
F16 = mybir.dt.float16
BF16 = mybir.dt.bfloat16
AF = mybir.ActivationFunctionType
ALU = mybir.AluOpType
AXL = mybir.AxisListType
BF = ml_dtypes.bfloat16

IN_ROWS = R + 6          # in tile rows (lo: r0-3 .., hi: r0-2 ..)
C1_ROWS = R + 4          # conv1 out rows [r0-2, r0+R+2)
CS_ROWS = R + 2          # csm rows [r0-1, r0+R+1)

# weight pack (f16): conv1 6x128 | conv2 9x128 | conv3 9x128 | 3 bias cols
W1C = 0
W2C = 768
W3C = 1920
BC = 3072
WCOLS = 3075


def build_program():
    nc = bacc.Bacc("TRN2", target_bir_lowering=False, debug=False,
                   num_devices=NCORES)

    x1d = nc.dram_tensor("x1pad", [64, 103 * WP], F16, kind="ExternalInput").ap()
    x2d = nc.dram_tensor("x2pad", [64, 103 * WP], F16, kind="ExternalInput").ap()
    xsd = nc.dram_tensor("xskip", [128, (HLOC + 2) * WP], F16, kind="ExternalInput").ap()
    ftd = nc.dram_tensor("fT", [192, HLOC * 128], BF16, kind="ExternalInput").ap()
    ffd = nc.dram_tensor("fF", [192, HLOC * 128], F16, kind="ExternalInput").ap()
    wpd = nc.dram_tensor("wpk", [128, WCOLS], F16, kind="ExternalInput").ap()
    emd = nc.dram_tensor("emask", [128, 2], F32, kind="ExternalInput").ap()
    wbd = nc.dram_tensor("wbias", [128, 3], F32, kind="ExternalInput").ap()
    oad = nc.dram_tensor("outA", [128, HLOC * 128], F32, kind="ExternalOutput").ap()
    obd = nc.dram_tensor("outB", [64, HLOC * 128], F32, kind="ExternalOutput").ap()
    import os
    dbg = os.environ.get("K2_DEBUG") == "1"
    if dbg:
        dc1 = nc.dram_tensor("dbg_c1", [128, 1 + C1_ROWS * WP + 4], F16,
                             kind="ExternalOutput").ap()
        dcs = nc.dram_tensor("dbg_cs", [128, 1 + CS_ROWS * WP + 4], F16,
                             kind="ExternalOutput").ap()
        dqt = nc.dram_tensor("dbg_qt", [128, R * 192], F16, kind="ExternalOutput").ap()
        dkt = nc.dram_tensor("dbg_kt", [128, R * 192], F16, kind="ExternalOutput").ap()
        dse = nc.dram_tensor("dbg_se", [128, 4 * 384], BF16, kind="ExternalOutput").ap()
        dg1 = nc.dram_tensor("dbg_g1", [128, 64], F32, kind="ExternalOutput").ap()
        dg1b = nc.dram_tensor("dbg_g1b", [64, 64], F32, kind="ExternalOutput").ap()
        dg2 = nc.dram_tensor("dbg_g2", [128, 64], F32, kind="ExternalOutput").ap()
        dg2b = nc.dram_tensor("dbg_g2b", [64, 64], F32, kind="ExternalOutput").ap()

    with tile.TileContext(nc) as tc, ExitStack() as ctx:
        P = lambda **kw: ctx.enter_context(tc.tile_pool(**kw))
        wpool = P(name="w", bufs=1)
        iop = P(name="io", bufs=2)
        xskp = P(name="xsk", bufs=2)
        c1p = P(name="c1", bufs=2)
        csp = P(name="cs", bufs=2)
        qkp = P(name="qk", bufs=2)
        ftp = P(name="ft", bufs=2)
        lrp = P(name="lr", bufs=2)
        sep = P(name="se", bufs=3)
        smp = P(name="sm", bufs=2)
        sm3 = P(name="sm3", bufs=3)
        pcv = P(name="pcv", bufs=2, space="PSUM")
        psc = P(name="psc", bufs=1, space="PSUM")
        pap = P(name="pap", bufs=1, space="PSUM")

        wb = wpool.tile([128, WCOLS], F16)
        nc.sync.dma_start(wb[:], wpd[:])
        em = wpool.tile([128, 2], F32)
        nc.sync.dma_start(em[:], emd[:])
        wbias = wpool.tile([128, 3], F32)
        nc.sync.dma_start(wbias[:], wbd[:])
        b1 = wbias[:, 0:1]
        b2 = wbias[:, 1:2]
        cb = wbias[:, 2:3]

        def rview(t, u, n):
            """[128, n, 192] data view of rows [u, u+n) of a guarded row tile."""
            return t[:, 1 + u * WP:1 + (u + n) * WP].rearrange(
                "p (r c) -> p r c", c=WP)[:, :, 1:193]

        def ps3(ps):
            return ps[:, 0:388].rearrange("p (r c) -> p r c", c=WP)[:, :, 1:193]

        def clear_pads(t, rows):
            nc.gpsimd.memset(t[:, 0:1], 0.0)
            v = t[:, 1:1 + rows * WP].rearrange("p (r c) -> p r c", c=WP)
            nc.gpsimd.memset(v[:, :, 0:1], 0.0)
            nc.gpsimd.memset(v[:, :, 193:194], 0.0)
            nc.gpsimd.memset(t[:, 1 + rows * WP:], 0.0)

        for blk in range(NBLK):
            r0 = blk * R

            # ---------------- DMAs ----------------
            in1 = iop.tile([128, 1 + IN_ROWS * WP + 4], F16, tag="in1")
            in2 = iop.tile([128, 1 + IN_ROWS * WP + 4], F16, tag="in2")
            for t, src in ((in1, x1d), (in2, x2d)):
                nc.gpsimd.memset(t[:, 0:1], 0.0)
                nc.gpsimd.memset(t[:, 1 + IN_ROWS * WP:], 0.0)
                nc.sync.dma_start(t[0:64, 1:1 + IN_ROWS * WP],
                                  src[:, r0 * WP:(r0 + IN_ROWS) * WP])
                nc.sync.dma_start(t[64:128, 1:1 + IN_ROWS * WP],
                                  src[:, (r0 + 1) * WP:(r0 + 1 + IN_ROWS) * WP])
            xs1 = xskp.tile([128, CS_ROWS * WP], F16, tag="xs1")
            xs2 = xskp.tile([128, CS_ROWS * WP], F16, tag="xs2")
            nc.sync.dma_start(xs1[0:64, :], xsd[0:64, r0 * WP:(r0 + CS_ROWS) * WP])
            nc.sync.dma_start(xs1[64:128, :], xsd[0:64, r0 * WP:(r0 + CS_ROWS) * WP])
            nc.sync.dma_start(xs2[0:64, :], xsd[64:128, r0 * WP:(r0 + CS_ROWS) * WP])
            nc.sync.dma_start(xs2[64:128, :], xsd[64:128, r0 * WP:(r0 + CS_ROWS) * WP])
            fta = ftp.tile([128, R * 128], BF16, tag="fta")
            ftb = ftp.tile([64, R * 128], BF16, tag="ftb")
            nc.sync.dma_start(fta[:], ftd[0:128, r0 * 128:(r0 + R) * 128])
            nc.sync.dma_start(ftb[:], ftd[128:192, r0 * 128:(r0 + R) * 128])
            ffa = ftp.tile([128, R * 128], F16, tag="ffa")
            ffb = ftp.tile([64, R * 128], F16, tag="ffb")
            nc.sync.dma_start(ffa[:], ffd[0:128, r0 * 128:(r0 + R) * 128])
            nc.sync.dma_start(ffb[:], ffd[128:192, r0 * 128:(r0 + R) * 128])

            # ---------------- conv1 ----------------
            # out rows [r0-2, r0+R+2), stream-packed [fe1 co | fe2 co]
            c1s = [c1p.tile([128, 1 + C1_ROWS * WP + 4], F16, tag=f"c1_{i}",
                            name=f"c1_{i}") for i in range(2)]
            for ii, (ind, dst) in enumerate(((in1, c1s[0]), (in2, c1s[1]))):
                clear_pads(dst, C1_ROWS)
                for y0 in range(r0 - 2, r0 + R + 2, 2):
                    ps = pcv.tile([128, 388], F32, tag="cv")
                    for mm in range(6):
                        dx = mm % 3
                        j = (y0 - 1 - (r0 - 3)) if mm < 3 else (y0 + 1 - (r0 - 3))
                        nc.tensor.matmul(
                            ps[:, 0:388],
                            wb[:, W1C + mm * 128:W1C + (mm + 1) * 128],
                            ind[:, j * WP + dx:j * WP + dx + 388],
                            start=(mm == 0), stop=(mm == 5))
                    u = y0 - (r0 - 2)
                    nc.vector.tensor_scalar(rview(dst, u, 2), ps3(ps),
                                            b1, 0.0, ALU.add, ALU.max)
                if blk == 0:
                    v = rview(dst, 0, 2)
                    nc.gpsimd.tensor_scalar_mul(v, v, em[:, 0:1])
                if blk == NBLK - 1:
                    v = rview(dst, C1_ROWS - 2, 2)
                    nc.gpsimd.tensor_scalar_mul(v, v, em[:, 1:2])

            # ---------------- conv2 + skip ----------------
            cst = [csp.tile([128, 1 + CS_ROWS * WP + 4], F16, tag=f"cs_{i}",
                            name=f"cs_{i}") for i in range(2)]
            for ii, (src, dst, xsk) in enumerate(((c1s[0], cst[0], xs1),
                                                  (c1s[1], cst[1], xs2))):
                clear_pads(dst, CS_ROWS)
                for y0 in range(r0 - 1, r0 + R + 1, 2):
                    ps = pcv.tile([128, 388], F32, tag="cv")
                    for mm in range(9):
                        dy, dx = divmod(mm, 3)
                        j = (y0 + dy - 1) - (r0 - 2)
                        nc.tensor.matmul(
                            ps[:, 0:388],
                            wb[:, W2C + mm * 128:W2C + (mm + 1) * 128],
                            src[:, j * WP + dx:j * WP + dx + 388],
                            start=(mm == 0), stop=(mm == 8))
                    u = y0 - (r0 - 1)
                    nc.vector.scalar_tensor_tensor(
                        rview(dst, u, 2), ps3(ps), b2,
                        xsk[:, u * WP:(u + 2) * WP].rearrange(
                            "p (r c) -> p r c", c=WP)[:, :, 1:193],
                        ALU.add, ALU.add)
                if blk == 0:
                    v = rview(dst, 0, 1)
                    nc.gpsimd.tensor_scalar_mul(v, v, em[:, 0:1])
                if blk == NBLK - 1:
                    v = rview(dst, CS_ROWS - 1, 1)
                    nc.gpsimd.tensor_scalar_mul(v, v, em[:, 1:2])

            # ---------------- conv3 -> Q/K (f16, stream-packed) ----------------
            qt = qkp.tile([128, R * 192], F16, tag="qt")
            kt = qkp.tile([128, R * 192], F16, tag="kt")
            for src, dst in ((cst[0], qt), (cst[1], kt)):
                for y0 in range(r0, r0 + R, 2):
                    ps = pcv.tile([128, 388], F32, tag="cv")
                    for mm in range(9):
                        dy, dx = divmod(mm, 3)
                        j = (y0 + dy - 1) - (r0 - 1)
                        nc.tensor.matmul(
                            ps[:, 0:388],
                            wb[:, W3C + mm * 128:W3C + (mm + 1) * 128],
                            src[:, j * WP + dx:j * WP + dx + 388],
                            start=(mm == 0), stop=(mm == 8))
                    u = y0 - r0
                    nc.scalar.activation(
                        dst[:, u * 192:(u + 2) * 192].rearrange(
                            "p (r c) -> p r c", c=192),
                        ps3(ps), AF.Identity, bias=cb)

            if dbg and blk == 0:
                nc.sync.dma_start(dc1[:], c1s[0][:])
                nc.sync.dma_start(dcs[:], cst[0][:])
                nc.sync.dma_start(dqt[:], qt[:])
                nc.sync.dma_start(dkt[:], kt[:])

            # ---------------- attention (3-deep row pipeline) ----------------
            otA = lrp.tile([128, R * 128], F32, tag="otA")
            otB = lrp.tile([64, R * 128], F32, tag="otB")
            specs = ((qt, kt, 0), (kt, qt, 0), (qt, kt, 64), (kt, qt, 64))
            rows = {}
            for it in range(R + 2):
                # -- scores + exp + normalization for row `it` --
                if it < R:
                    st = {}
                    qb = it * 192
                    fb = it * 128
                    st["psS"] = [psc.tile([128, 384], F32, tag=f"sc{s}",
                                          name=f"sc{s}") for s in range(4)]
                    for s, (LS, RS, h) in enumerate(specs):
                        nc.tensor.matmul(st["psS"][s][:, 0:192],
                                         LS[h:h + 64, qb:qb + 128],
                                         RS[h:h + 64, qb:qb + 192],
                                         start=True, stop=True)
                    for s, (LS, RS, h) in enumerate(specs):
                        nc.tensor.matmul(st["psS"][s][0:64, 192:384],
                                         LS[h:h + 64, qb + 128:qb + 192],
                                         RS[h:h + 64, qb:qb + 192],
                                         start=True, stop=True)
                    st["se"] = [sep.tile([128, 384], BF16, tag=f"se{s}",
                                         name=f"se{s}") for s in range(4)]
                    for s in range(4):
                        nc.scalar.activation(st["se"][s][:], st["psS"][s][:],
                                             AF.Exp)
                    st["za"] = sm3.tile([128, 2], F32, tag="za")
                    st["zb"] = sm3.tile([64, 2], F32, tag="zb")
                    nc.vector.tensor_reduce(st["za"][:, 0:1],
                                            st["se"][0][:, 0:192], AXL.X, ALU.add)
                    nc.vector.tensor_reduce(st["za"][:, 1:2],
                                            st["se"][2][:, 0:192], AXL.X, ALU.add)
                    nc.vector.tensor_reduce(st["zb"][:, 0:1],
                                            st["se"][0][0:64, 192:384], AXL.X, ALU.add)
                    nc.vector.tensor_reduce(st["zb"][:, 1:2],
                                            st["se"][2][0:64, 192:384], AXL.X, ALU.add)
                    st["iza"] = sm3.tile([128, 2], F32, tag="iza")
                    st["izb"] = sm3.tile([64, 2], F32, tag="izb")
                    nc.vector.reciprocal(st["iza"][:], st["za"][:])
                    nc.vector.reciprocal(st["izb"][:], st["zb"][:])
                    st["fs1"] = smp.tile([128, 64], BF16, tag="fs1")
                    st["fs1b"] = smp.tile([64, 64], BF16, tag="fs1b")
                    nc.gpsimd.tensor_scalar_mul(st["fs1"][:], fta[:, fb:fb + 64],
                                                st["iza"][:, 0:1])
                    nc.gpsimd.tensor_scalar_mul(st["fs1b"][:], ftb[:, fb:fb + 64],
                                                st["izb"][:, 0:1])
                    if dbg and blk == 0 and it == 0:
                        for s_ in range(4):
                            nc.sync.dma_start(dse[:, s_ * 384:(s_ + 1) * 384],
                                              st["se"][s_][:])
                    rows[it] = st

                # -- apply matmuls: stage1 = rows[it-1] (se0/se1),
                #    stage2 = rows[it-2] (se2/se3), 4 uniform classes --
                work = []
                s1 = rows.get(it - 1)
                s2 = rows.get(it - 2)
                pga = pap.tile([128, 512], F32, tag="pga", name="pga")
                pgb = pap.tile([128, 512], F32, tag="pgb", name="pgb")
                # per group g: regions in tile (pga: g0/g1, pgb: g2/g3),
                # base=(g%2)*256: [M128A | M128B | M64A | M64B] x 64 cols
                pgr = [(pga, 0), (pga, 256), (pgb, 0), (pgb, 256)]
                if s1 is not None:
                    fb1 = (it - 1) * 128
                    work.append((0, s1["se"][0], s1["fs1"][:], s1["fs1b"][:]))
                    work.append((1, s1["se"][1], fta[:, fb1 + 64:fb1 + 128],
                                 ftb[:, fb1 + 64:fb1 + 128]))
                if s2 is not None:
                    work.append((2, s2["se"][2], s2["g1s"][:], s2["g1sb"][:]))
                    work.append((3, s2["se"][3], s2["g2c"][:], s2["g2cb"][:]))
                for g, se_t, rA, rB in work:    # (K=128, M=128, N=64)
                    t, base = pgr[g]
                    nc.tensor.matmul(t[:, base:base + 64], se_t[0:128, 0:128],
                                     rA, start=True, stop=True)
                for g, se_t, rA, rB in work:    # (K=128, M=64, N=64)
                    t, base = pgr[g]
                    nc.tensor.matmul(t[0:64, base + 128:base + 192],
                                     se_t[0:128, 128:192], rA,
                                     start=True, stop=True)
                for g, se_t, rA, rB in work:    # (K=64, M=128, N=64)
                    t, base = pgr[g]
                    nc.tensor.matmul(t[:, base + 64:base + 128],
                                     se_t[0:64, 192:320], rB,
                                     start=True, stop=True)
                for g, se_t, rA, rB in work:    # (K=64, M=64, N=64)
                    t, base = pgr[g]
                    nc.tensor.matmul(t[0:64, base + 192:base + 256],
                                     se_t[0:64, 320:384], rB,
                                     start=True, stop=True)

                # -- post: g1/g2 (+ scaled copies) for row it-1 --
                if s1 is not None:
                    fb1 = (it - 1) * 128
                    s1["g1"] = sm3.tile([128, 64], F32, tag="g1", name="g1")
                    s1["g1b"] = sm3.tile([64, 64], F32, tag="g1b", name="g1b")
                    s1["g2"] = sm3.tile([128, 64], F32, tag="g2", name="g2")
                    s1["g2b"] = sm3.tile([64, 64], F32, tag="g2b", name="g2b")
                    # g1 = (pg1_A + pg1_B) * iza0 + low1 ; pg1 regions at base 256
                    nc.vector.scalar_tensor_tensor(
                        s1["g1"][:], pga[:, 256:320], s1["iza"][:, 0:1],
                        ffa[:, fb1:fb1 + 64], ALU.mult, ALU.add)
                    nc.vector.scalar_tensor_tensor(
                        s1["g1"][:], pga[:, 320:384], s1["iza"][:, 0:1],
                        s1["g1"][:], ALU.mult, ALU.add)
                    nc.vector.scalar_tensor_tensor(
                        s1["g1b"][:], pga[0:64, 384:448], s1["izb"][:, 0:1],
                        ffb[:, fb1:fb1 + 64], ALU.mult, ALU.add)
                    nc.vector.scalar_tensor_tensor(
                        s1["g1b"][:], pga[0:64, 448:512], s1["izb"][:, 0:1],
                        s1["g1b"][:], ALU.mult, ALU.add)
                    # g2 = pg0_A + pg0_B + low2 ; pg0 regions at base 0
                    nc.vector.tensor_add(s1["g2"][:], pga[:, 0:64],
                                         ffa[:, fb1 + 64:fb1 + 128])
                    nc.vector.tensor_add(s1["g2"][:], pga[:, 64:128], s1["g2"][:])
                    nc.vector.tensor_add(s1["g2b"][:], pga[0:64, 128:192],
                                         ffb[:, fb1 + 64:fb1 + 128])
                    nc.vector.tensor_add(s1["g2b"][:], pga[0:64, 192:256],
                                         s1["g2b"][:])
                    s1["g1s"] = smp.tile([128, 64], BF16, tag="g1s", name="g1s")
                    s1["g1sb"] = smp.tile([64, 64], BF16, tag="g1sb", name="g1sb")
                    s1["g2c"] = smp.tile([128, 64], BF16, tag="g2c", name="g2c")
                    s1["g2cb"] = smp.tile([64, 64], BF16, tag="g2cb", name="g2cb")
                    nc.gpsimd.tensor_scalar_mul(s1["g1s"][:], s1["g1"][:],
                                                s1["iza"][:, 1:2])
                    nc.gpsimd.tensor_scalar_mul(s1["g1sb"][:], s1["g1b"][:],
                                                s1["izb"][:, 1:2])
                    nc.gpsimd.tensor_copy(s1["g2c"][:], s1["g2"][:])
                    nc.gpsimd.tensor_copy(s1["g2cb"][:], s1["g2b"][:])
                    if dbg and blk == 0 and it == 1:
                        nc.sync.dma_start(dg1[:], s1["g1"][:])
                        nc.sync.dma_start(dg1b[:], s1["g1b"][:])
                        nc.sync.dma_start(dg2[:], s1["g2"][:])
                        nc.sync.dma_start(dg2b[:], s1["g2b"][:])

                # -- final combines for row it-2 --
                if s2 is not None:
                    fb2 = (it - 2) * 128
                    # left = g1 + (pg3_A + pg3_B) * iza2 ; pg3 at pgb base 256
                    nc.vector.scalar_tensor_tensor(
                        otA[:, fb2:fb2 + 64], pgb[:, 256:320], s2["iza"][:, 1:2],
                        s2["g1"][:], ALU.mult, ALU.add)
                    nc.vector.scalar_tensor_tensor(
                        otA[:, fb2:fb2 + 64], pgb[:, 320:384], s2["iza"][:, 1:2],
                        otA[:, fb2:fb2 + 64], ALU.mult, ALU.add)
                    nc.vector.scalar_tensor_tensor(
                        otB[:, fb2:fb2 + 64], pgb[0:64, 384:448], s2["izb"][:, 1:2],
                        s2["g1b"][:], ALU.mult, ALU.add)
                    nc.vector.scalar_tensor_tensor(
                        otB[:, fb2:fb2 + 64], pgb[0:64, 448:512], s2["izb"][:, 1:2],
                        otB[:, fb2:fb2 + 64], ALU.mult, ALU.add)
                    # right = g2 + pg2_A + pg2_B ; pg2 at pgb base 0
                    nc.vector.tensor_add(otA[:, fb2 + 64:fb2 + 128],
                                         pgb[:, 0:64], s2["g2"][:])
                    nc.vector.tensor_add(otA[:, fb2 + 64:fb2 + 128],
                                         pgb[:, 64:128], otA[:, fb2 + 64:fb2 + 128])
                    nc.vector.tensor_add(otB[:, fb2 + 64:fb2 + 128],
                                         pgb[0:64, 128:192], s2["g2b"][:])
                    nc.vector.tensor_add(otB[:, fb2 + 64:fb2 + 128],
                                         pgb[0:64, 192:256], otB[:, fb2 + 64:fb2 + 128])
                    del rows[it - 2]

            nc.sync.dma_start(oad[:, r0 * 128:(r0 + R) * 128], otA[:])
            nc.sync.dma_start(obd[:, r0 * 128:(r0 + R) * 128], otB[:])

    nc.compile()
    return nc


# ----------------------------------------------------------------------------
# host-side prep

def _pack_weights(fe1_w1, fe1_b1, fe1_w2, fe1_b2, fe2_w1, fe2_b1, fe2_w2,
                  fe2_b2, conv_w, conv_b):
    wpk = np.zeros((128, WCOLS), np.float32)
    w1 = (np.asarray(fe1_w1, np.float32), np.asarray(fe2_w1, np.float32))
    w2 = (np.asarray(fe1_w2, np.float32), np.asarray(fe2_w2, np.float32))
    w3 = np.asarray(conv_w, np.float32)
    # conv1: mm 0..2 = dy01 pairs per dx, mm 3..5 = dy2 (upper half zero)
    for mm in range(6):
        dx = mm % 3
        col = W1C + mm * 128
        for ws in range(2):
            if mm < 3:
                for dy in range(2):
                    wpk[dy * 64:(dy + 1) * 64, col + ws * 64:col + ws * 64 + 64] = \
                        w1[ws][:, :, dy, dx].T
            else:
                wpk[0:64, col + ws * 64:col + ws * 64 + 64] = w1[ws][:, :, 2, dx].T
    # conv2 / conv3: block-diagonal over streams, 9 taps
    for mm in range(9):
        dy, dx = divmod(mm, 3)
        col = W2C + mm * 128
        wpk[0:64, col:col + 64] = w2[0][:, :, dy, dx].T
        wpk[64:128, col + 64:col + 128] = w2[1][:, :, dy, dx].T
        col = W3C + mm * 128
        wpk[0:64, col:col + 64] = w3[:, :, dy, dx].T
        wpk[64:128, col + 64:col + 128] = w3[:, :, dy, dx].T
    wbias = np.zeros((128, 3), np.float32)
    wbias[0:64, 0] = fe1_b1
    wbias[64:128, 0] = fe2_b1
    wbias[0:64, 1] = fe1_b2
    wbias[64:128, 1] = fe2_b2
    wbias[0:64, 2] = conv_b
    wbias[64:128, 2] = conv_b
    return wpk.astype(np.float16), wbias


def _pad_rows(x, lo, hi):
    """rows [lo, hi) of x[64, H, W], zero fill OOB rows, width pad to 194."""
    n = hi - lo
    out = np.zeros((64, n, WP), np.float32)
    clo, chi = max(lo, 0), min(hi, H)
    if chi > clo:
        out[:, clo - lo:chi - lo, 1:193] = x[:, clo:chi, :]
    return out


def _prep_core(low1, low2, b, h0):
    x1 = _pad_rows(low1[b], h0 - 3, h0 + 100).reshape(64, -1).astype(np.float16)
    x2 = _pad_rows(low2[b], h0 - 3, h0 + 100).reshape(64, -1).astype(np.float16)
    xs = np.concatenate([_pad_rows(low1[b], h0 - 1, h0 + HLOC + 1),
                         _pad_rows(low2[b], h0 - 1, h0 + HLOC + 1)],
                        axis=0).reshape(128, -1).astype(np.float16)
    ft = np.concatenate([low1[b][:, h0:h0 + HLOC, :],
                         low2[b][:, h0:h0 + HLOC, :]], axis=0)  # [128, 96, 192]
    ftf = np.ascontiguousarray(ft.transpose(2, 1, 0)).reshape(192, HLOC * 128)
    em = np.empty((128, 2), np.float32)
    em[:, 0] = 1.0 if h0 > 0 else 0.0    # rows -2,-1 valid only for h0=96
    em[:, 1] = 1.0 if h0 == 0 else 0.0   # rows 96,97 valid only for h0=0
    return {"x1pad": x1, "x2pad": x2, "xskip": xs,
            "fT": ftf.astype(BF), "fF": ftf.astype(np.float16),
            "emask": em.astype(np.float32)}


_cached = {}


def _get_program():
    if "nc" not in _cached:
        _cached["nc"] = build_program()
    return _cached["nc"]


def run(inputs, trace=False):
    from concourse.bass_utils import run_bass_kernel_spmd

    wpk, wbias = _pack_weights(
        inputs["fe1_w1"], inputs["fe1_b1"], inputs["fe1_w2"], inputs["fe1_b2"],
        inputs["fe2_w1"], inputs["fe2_b1"], inputs["fe2_w2"], inputs["fe2_b2"],
        inputs["conv_w"], inputs["conv_b"])
    low1 = np.asarray(inputs["low1"], np.float32)
    low2 = np.asarray(inputs["low2"], np.float32)
    in_maps = []
    for core in range(NCORES):
        b, h0 = core // 2, (core % 2) * HLOC
        m = _prep_core(low1, low2, b, h0)
        m["wpk"] = wpk
        m["wbias"] = wbias
        in_maps.append(m)

    nc = _get_program()
    res = run_bass_kernel_spmd(nc, in_maps, list(range(NCORES)), trace=trace)

    left = np.empty((B, C, H, W), np.float32)
    right = np.empty((B, C, H, W), np.float32)
    for core in range(NCORES):
        b, h0 = core // 2, (core % 2) * HLOC
        oa = res.results[core]["outA"].reshape(128, HLOC, 128)  # [w, hc, c2]
        ob = res.results[core]["outB"].reshape(64, HLOC, 128)
        left[b, :, h0:h0 + HLOC, 0:128] = oa[:, :, 0:64].transpose(2, 1, 0)
        right[b, :, h0:h0 + HLOC, 0:128] = oa[:, :, 64:128].transpose(2, 1, 0)
        left[b, :, h0:h0 + HLOC, 128:192] = ob[:, :, 0:64].transpose(2, 1, 0)
        right[b, :, h0:h0 + HLOC, 128:192] = ob[:, :, 64:128].transpose(2, 1, 0)
    return (left, right), res


def kernel(**inputs):
    (left, right), _ = run(inputs)
    return (left, right)
